# revision 6
# baseline (speedup 1.0000x reference)
"""AEKD-Teacher ensemble-vote kernel for 8 Trainium2 NeuronCores.

Per sample: argmax each of 4 models' logits over C=1000, majority vote with
uniform-random tie-break among max-vote models; output = chosen model's row,
or the mean of all 4 rows when the max vote count is shared by >1 class.

Observation: output row == sum_m w[m]*x_m[row] with per-sample weights
(0.25 each on tie, one-hot on the selected model otherwise), so the kernel is
one fused pass: argmax (DVE) -> votes (Pool) -> diag-weight matmul accumulate
(PE/PSUM) -> copy out (ACT).  Pure data-parallel over N=32768 samples.
"""

import numpy as np

import concourse.bass as bass
import concourse.mybir as mybir
import concourse.tile as tile
from concourse.vector_clock import ScopedClock

# ---------------------------------------------------------------------------
# Workarounds for this container's walrus: only ONE sync-wait per instruction.
_MAXW = 1


def _patched_drain_and_barrier(self, tick_clock, wait_clock):
    drain_inst = self.nc.sync.drain()
    wait_clock.add_sem_waits(
        drain_inst.ins, ScopedClock({None: tick_clock.global_clock})
    )
    si = drain_inst.ins.sync_info
    waits = list(si.on_wait)
    if len(waits) > _MAXW:
        drain_inst.ins.sync_info = mybir.SyncInfo(
            on_wait=waits[:_MAXW], on_update=list(si.on_update)
        )
        for i in range(_MAXW, len(waits), _MAXW):
            nop = self.nc.sync.nop(nofuse=True)
            nop.ins.sync_info = mybir.SyncInfo(
                on_wait=waits[i : i + _MAXW], on_update=[]
            )
    self.nc.all_engine_barrier()
    assert self.sems is not None
    popped = self.nc._tile_sem_poison_stack.pop()
    assert popped is self._sem_poison
    self.nc.clear_and_free_semaphores(list(self.sems.allocated().values()))
    self.nc.all_engine_barrier()


tile.TileContext._drain_and_barrier = _patched_drain_and_barrier

_nop_counter = [0]


def _split_multi_waits(nc):
    for f in nc.m.functions:
        for bb in f.blocks:
            insts = list(bb.instructions)
            if not any(
                i.sync_info is not None and len(i.sync_info.on_wait) > 1
                for i in insts
            ):
                continue
            new_insts = []
            for inst in insts:
                si = inst.sync_info
                if si is not None and len(si.on_wait) > 1:
                    waits = list(si.on_wait)
                    for w in waits[:-1]:
                        _nop_counter[0] += 1
                        nop = mybir.InstNoOp(name=f"mw-nop-{_nop_counter[0]}")
                        nop.engine = inst.engine
                        nop.sync_info = mybir.SyncInfo(on_wait=[w], on_update=[])
                        new_insts.append(nop)
                    inst.sync_info = mybir.SyncInfo(
                        on_wait=[waits[-1]], on_update=list(si.on_update)
                    )
                new_insts.append(inst)
            bb.instructions = new_insts


# ---------------------------------------------------------------------------
N_CORES = 8
N, C = 32768, 1000
NS = N // N_CORES          # samples per core
P = 128                    # partitions / samples per tile
NT = NS // P               # 32 tiles per core
G = 4                      # tiles per vote group
NG = NT // G               # 8 groups per core
M = 4                      # models
GW = M * G                 # label/weight columns per group
ALU = mybir.AluOpType
F32 = mybir.dt.float32
AF = mybir.ActivationFunctionType


def _build_graph(reps=1):
    nc = bass.Bass()
    x_ext = [
        nc.declare_dram_parameter(f"x{m}", [NS, C], F32, isOutput=False)
        for m in range(M)
    ]
    r_ext = nc.declare_dram_parameter("r", [P, NG * GW], F32, isOutput=False)
    out_ext = nc.declare_dram_parameter("out", [NS, C], F32, isOutput=True)

    with tile.TileContext(nc) as tc:
        with (
            tc.tile_pool(name="const", bufs=1) as cpool,
            tc.tile_pool(name="x", bufs=1) as xpool,
            tc.tile_pool(name="work", bufs=1) as wpool,
            tc.tile_pool(name="votes", bufs=1) as vpool,
            tc.tile_pool(name="outp", bufs=1) as opool,
            tc.tile_pool(name="psum", bufs=3, space="PSUM") as ppool,
        ):
            # constants
            rt = cpool.tile([P, NG * GW], F32, tag="rt")
            nc.sync.dma_start(rt[:], r_ext[:])
            ioi = cpool.tile([P, C], mybir.dt.int32, tag="ioi")
            nc.gpsimd.iota(ioi[:], pattern=[[1, C]], base=0, channel_multiplier=0)
            # descending iota: 1000 - j  (label enc = 1000 - argmax, equality-preserving)
            iod = cpool.tile([P, C], F32, tag="iod")
            nc.vector.tensor_scalar(
                iod[:], ioi[:], -1.0, float(C), op0=ALU.mult, op1=ALU.add
            )
            io2 = cpool.tile([P, P], mybir.dt.int32, tag="io2")
            nc.gpsimd.iota(io2[:], pattern=[[1, P]], base=0, channel_multiplier=-1)
            ident = cpool.tile([P, P], F32, tag="ident")
            nc.gpsimd.tensor_scalar(ident[:], io2[:], 0.0, None, op0=ALU.is_equal)

            for g in [g for _rep in range(reps) for g in range(NG)]:
                lab = vpool.tile([P, GW], F32, tag="lab", bufs=2)
                xs = [[None] * M for _ in range(G)]
                for j in range(G):
                    t = g * G + j
                    for m in range(M):
                        xt = xpool.tile([P, C], F32, tag=f"x{m}", bufs=G + 2)
                        nc.sync.dma_start(xt[:], x_ext[m][t * P : (t + 1) * P, :])
                        xs[j][m] = xt
                        mx = wpool.tile([P, 1], F32, tag="mx", bufs=8)
                        scr = wpool.tile([P, C], F32, tag="scr", bufs=3)
                        nc.vector.tensor_scalar(
                            scr[:], xt[:], 1.0, None,
                            op0=ALU.mult, op1=ALU.max, accum_out=mx[:],
                        )
                        scr2 = wpool.tile([P, C], F32, tag="scr2", bufs=3)
                        nc.vector.scalar_tensor_tensor(
                            scr2[:], xt[:], mx[:], iod[:],
                            op0=ALU.is_ge, op1=ALU.mult,
                            accum_out=lab[:, m * G + j : m * G + j + 1],
                        )

                # ---- votes on Pool: per-model [P, G] slices
                lm = [lab[:, m * G : (m + 1) * G] for m in range(M)]
                rm = [
                    rt[:, g * GW + m * G : g * GW + (m + 1) * G] for m in range(M)
                ]

                def vtile(tag, bufs=2):
                    return vpool.tile([P, G], F32, tag=tag, bufs=bufs, name=tag)

                eqs = {}
                for i in range(M):
                    for j2 in range(i + 1, M):
                        e = vtile(f"eq{i}{j2}")
                        nc.vector.tensor_tensor(e[:], lm[i][:], lm[j2][:], op=ALU.is_equal)
                        eqs[(i, j2)] = eqs[(j2, i)] = e
                cnt = []
                for i in range(M):
                    others = [eqs[(i, k)] for k in range(M) if k != i]
                    c01 = vtile(f"c{i}a")
                    nc.gpsimd.tensor_tensor(c01[:], others[0][:], others[1][:], op=ALU.add)
                    ci = vtile(f"c{i}")
                    nc.gpsimd.tensor_tensor(ci[:], c01[:], others[2][:], op=ALU.add)
                    cnt.append(ci)
                m01 = vtile("m01")
                nc.vector.tensor_tensor(m01[:], cnt[0][:], cnt[1][:], op=ALU.max)
                m23 = vtile("m23")
                nc.vector.tensor_tensor(m23[:], cnt[2][:], cnt[3][:], op=ALU.max)
                mx3 = vtile("mx3")
                nc.vector.tensor_tensor(mx3[:], m01[:], m23[:], op=ALU.max)
                ism = []
                for i in range(M):
                    e = vtile(f"ism{i}")
                    nc.vector.tensor_tensor(e[:], cnt[i][:], mx3[:], op=ALU.is_equal)
                    ism.append(e)
                s01 = vtile("s01")
                nc.gpsimd.tensor_tensor(s01[:], ism[0][:], ism[1][:], op=ALU.add)
                s23 = vtile("s23")
                nc.gpsimd.tensor_tensor(s23[:], ism[2][:], ism[3][:], op=ALU.add)
                s = vtile("s")
                nc.gpsimd.tensor_tensor(s[:], s01[:], s23[:], op=ALU.add)
                mxp1 = vtile("mxp1")
                nc.gpsimd.tensor_scalar(mxp1[:], mx3[:], 1.0, None, op0=ALU.add)
                tie = vtile("tie")
                nc.vector.tensor_tensor(tie[:], s[:], mxp1[:], op=ALU.not_equal)
                notie = vtile("notie")
                nc.gpsimd.tensor_scalar(
                    notie[:], tie[:], -1.0, 1.0, op0=ALU.mult, op1=ALU.add
                )
                tie4 = vtile("tie4")
                nc.gpsimd.tensor_scalar(tie4[:], tie[:], 0.25, None, op0=ALU.mult)
                us = []
                for i in range(M):
                    u = vtile(f"u{i}")
                    nc.vector.scalar_tensor_tensor(
                        u[:], rm[i][:], 1.0, ism[i][:], op0=ALU.add, op1=ALU.mult
                    )
                    us.append(u)
                b01 = vtile("b01")
                nc.vector.tensor_tensor(b01[:], us[0][:], us[1][:], op=ALU.max)
                b23 = vtile("b23")
                nc.vector.tensor_tensor(b23[:], us[2][:], us[3][:], op=ALU.max)
                b = vtile("b")
                nc.vector.tensor_tensor(b[:], b01[:], b23[:], op=ALU.max)
                W = vpool.tile([P, GW], F32, tag="W", bufs=2)
                for i in range(M):
                    oh = vtile(f"oh{i}")
                    nc.vector.tensor_tensor(oh[:], us[i][:], b[:], op=ALU.is_equal)
                    ohn = vtile(f"ohn{i}")
                    nc.gpsimd.tensor_tensor(ohn[:], oh[:], notie[:], op=ALU.mult)
                    nc.gpsimd.tensor_tensor(
                        W[:, i * G : (i + 1) * G], ohn[:], tie4[:], op=ALU.add
                    )

                # ---- weighted sum via PE diag matmuls
                for j in range(G):
                    t = g * G + j
                    diags = []
                    for m in range(M):
                        d = wpool.tile([P, P], F32, tag="diag", bufs=8)
                        nc.gpsimd.tensor_scalar(
                            d[:], ident[:],
                            W[:, m * G + j : m * G + j + 1], None, op0=ALU.mult,
                        )
                        diags.append(d)
                    ps = ppool.tile([P, C], F32, tag="ps")
                    for lo, hi in ((0, 512), (512, C)):
                        for m in range(M):
                            nc.tensor.matmul(
                                ps[:, lo:hi], diags[m][:], xs[j][m][:, lo:hi],
                                start=(m == 0), stop=(m == M - 1),
                            )
                    ot = opool.tile([P, C], F32, tag="ot", bufs=3)
                    nc.scalar.activation(ot[:], ps[:], AF.Copy)
                    nc.sync.dma_start(out_ext[t * P : (t + 1) * P, :], ot[:])

    _split_multi_waits(nc)
    return nc


_GRAPH = None


def _get_graph():
    global _GRAPH
    if _GRAPH is None:
        _GRAPH = _build_graph()
    return _GRAPH


def _tie_break_randoms():
    """Reproduce the reference's jax.random.uniform(key(42), (4, N)).

    Must run on the same jax backend as the reference: the neuron backend's
    PRNG stream differs from the CPU backend's.
    """
    import jax

    r = np.asarray(jax.random.uniform(jax.random.key(42), (M, N), dtype="float32"))
    # core layout: r_core[p, g*GW + m*G + j] = r[m, core*NS + (g*G+j)*P + p]
    rc = r.reshape(M, N_CORES, NG, G, P)          # [m, core, g, j, p]
    rc = rc.transpose(1, 4, 2, 0, 3)              # [core, p, g, m, j]
    return np.ascontiguousarray(rc.reshape(N_CORES, P, NG * GW).astype(np.float32))


def kernel(outputs1, outputs2, outputs3, outputs4):
    xs = [
        np.ascontiguousarray(np.asarray(o, dtype=np.float32))
        for o in (outputs1, outputs2, outputs3, outputs4)
    ]
    rcs = _tie_break_randoms()
    in_maps = []
    for c in range(N_CORES):
        im = {f"x{m}": xs[m][c * NS : (c + 1) * NS] for m in range(M)}
        im["r"] = rcs[c]
        in_maps.append(im)

    nc = _get_graph()
    from concourse.bass_utils import run_bass_kernel_spmd

    res = run_bass_kernel_spmd(nc, in_maps, core_ids=list(range(N_CORES)))
    return np.concatenate([res.results[c]["out"] for c in range(N_CORES)], axis=0)


# revision 7
# speedup vs baseline: 1.0245x; 1.0245x over previous
"""AEKD-Teacher ensemble-vote kernel for 8 Trainium2 NeuronCores.

Per sample: argmax each of 4 models' logits over C=1000, majority vote with
uniform-random tie-break among max-vote models; output = chosen model's row,
or the mean of all 4 rows when the max vote count is shared by >1 class.

Observation: output row == sum_m w[m]*x_m[row] with per-sample weights
(0.25 each on tie, one-hot on the selected model otherwise), so the kernel is
one fused pass: argmax (DVE) -> votes (Pool) -> diag-weight matmul accumulate
(PE/PSUM) -> copy out (ACT).  Pure data-parallel over N=32768 samples.
"""

import numpy as np

import concourse.bass as bass
import concourse.mybir as mybir
import concourse.tile as tile
from concourse.vector_clock import ScopedClock

# ---------------------------------------------------------------------------
# Workarounds for this container's walrus: only ONE sync-wait per instruction.
_MAXW = 1


def _patched_drain_and_barrier(self, tick_clock, wait_clock):
    drain_inst = self.nc.sync.drain()
    wait_clock.add_sem_waits(
        drain_inst.ins, ScopedClock({None: tick_clock.global_clock})
    )
    si = drain_inst.ins.sync_info
    waits = list(si.on_wait)
    if len(waits) > _MAXW:
        drain_inst.ins.sync_info = mybir.SyncInfo(
            on_wait=waits[:_MAXW], on_update=list(si.on_update)
        )
        for i in range(_MAXW, len(waits), _MAXW):
            nop = self.nc.sync.nop(nofuse=True)
            nop.ins.sync_info = mybir.SyncInfo(
                on_wait=waits[i : i + _MAXW], on_update=[]
            )
    self.nc.all_engine_barrier()
    assert self.sems is not None
    popped = self.nc._tile_sem_poison_stack.pop()
    assert popped is self._sem_poison
    self.nc.clear_and_free_semaphores(list(self.sems.allocated().values()))
    self.nc.all_engine_barrier()


tile.TileContext._drain_and_barrier = _patched_drain_and_barrier

_nop_counter = [0]


def _split_multi_waits(nc):
    for f in nc.m.functions:
        for bb in f.blocks:
            insts = list(bb.instructions)
            if not any(
                i.sync_info is not None and len(i.sync_info.on_wait) > 1
                for i in insts
            ):
                continue
            new_insts = []
            for inst in insts:
                si = inst.sync_info
                if si is not None and len(si.on_wait) > 1:
                    waits = list(si.on_wait)
                    for w in waits[:-1]:
                        _nop_counter[0] += 1
                        nop = mybir.InstNoOp(name=f"mw-nop-{_nop_counter[0]}")
                        nop.engine = inst.engine
                        nop.sync_info = mybir.SyncInfo(on_wait=[w], on_update=[])
                        new_insts.append(nop)
                    inst.sync_info = mybir.SyncInfo(
                        on_wait=[waits[-1]], on_update=list(si.on_update)
                    )
                new_insts.append(inst)
            bb.instructions = new_insts


# ---------------------------------------------------------------------------
N_CORES = 8
N, C = 32768, 1000
NS = N // N_CORES          # samples per core
P = 128                    # partitions / samples per tile
NT = NS // P               # 32 tiles per core
G = 4                      # tiles per vote group
NG = NT // G               # 8 groups per core
M = 4                      # models
GW = M * G                 # label/weight columns per group
ALU = mybir.AluOpType
F32 = mybir.dt.float32
BF16 = mybir.dt.bfloat16
AF = mybir.ActivationFunctionType


def _build_graph(reps=1):
    nc = bass.Bass()
    x_ext = [
        nc.declare_dram_parameter(f"x{m}", [NS, C], F32, isOutput=False)
        for m in range(M)
    ]
    r_ext = nc.declare_dram_parameter("r", [P, NG * GW], F32, isOutput=False)
    out_ext = nc.declare_dram_parameter("out", [NS, C], F32, isOutput=True)

    with tile.TileContext(nc) as tc:
        with (
            tc.tile_pool(name="const", bufs=1) as cpool,
            tc.tile_pool(name="x", bufs=1) as xpool,
            tc.tile_pool(name="work", bufs=1) as wpool,
            tc.tile_pool(name="votes", bufs=1) as vpool,
            tc.tile_pool(name="outp", bufs=1) as opool,
            tc.tile_pool(name="psum", bufs=3, space="PSUM") as ppool,
        ):
            # constants
            rt = cpool.tile([P, NG * GW], F32, tag="rt")
            nc.sync.dma_start(rt[:], r_ext[:])
            ioi = cpool.tile([P, C], mybir.dt.int32, tag="ioi")
            nc.gpsimd.iota(ioi[:], pattern=[[1, C]], base=0, channel_multiplier=0)
            # descending iota: 1000 - j  (label enc = 1000 - argmax, equality-preserving)
            iod = cpool.tile([P, C], F32, tag="iod")
            nc.vector.tensor_scalar(
                iod[:], ioi[:], -1.0, float(C), op0=ALU.mult, op1=ALU.add
            )
            io2 = cpool.tile([P, P], mybir.dt.int32, tag="io2")
            nc.gpsimd.iota(io2[:], pattern=[[1, P]], base=0, channel_multiplier=-1)
            ident = cpool.tile([P, P], F32, tag="ident")
            nc.gpsimd.tensor_scalar(ident[:], io2[:], 0.0, None, op0=ALU.is_equal)

            for g in [g for _rep in range(reps) for g in range(NG)]:
                lab = vpool.tile([P, GW], F32, tag="lab", bufs=2)
                xb = [[None] * M for _ in range(G)]
                for j in range(G):
                    t = g * G + j
                    for m in range(M):
                        xt = xpool.tile([P, C], F32, tag=f"x{m}", bufs=3)
                        nc.sync.dma_start(xt[:], x_ext[m][t * P : (t + 1) * P, :])
                        mx = wpool.tile([P, 1], F32, tag="mx", bufs=8)
                        # bf16 copy for the PE comes free as this op's out
                        xc = xpool.tile([P, C], BF16, tag=f"xb{m}", bufs=G + 2)
                        nc.vector.tensor_scalar(
                            xc[:], xt[:], 1.0, None,
                            op0=ALU.mult, op1=ALU.max, accum_out=mx[:],
                        )
                        xb[j][m] = xc
                        scr2 = wpool.tile([P, C], BF16, tag="scr2", bufs=3)
                        nc.vector.scalar_tensor_tensor(
                            scr2[:], xt[:], mx[:], iod[:],
                            op0=ALU.is_ge, op1=ALU.mult,
                            accum_out=lab[:, m * G + j : m * G + j + 1],
                        )

                # ---- votes on Pool: per-model [P, G] slices
                lm = [lab[:, m * G : (m + 1) * G] for m in range(M)]
                rm = [
                    rt[:, g * GW + m * G : g * GW + (m + 1) * G] for m in range(M)
                ]

                def vtile(tag, bufs=2):
                    return vpool.tile([P, G], F32, tag=tag, bufs=bufs, name=tag)

                eqs = {}
                for i in range(M):
                    for j2 in range(i + 1, M):
                        e = vtile(f"eq{i}{j2}")
                        nc.vector.tensor_tensor(e[:], lm[i][:], lm[j2][:], op=ALU.is_equal)
                        eqs[(i, j2)] = eqs[(j2, i)] = e
                cnt = []
                for i in range(M):
                    others = [eqs[(i, k)] for k in range(M) if k != i]
                    c01 = vtile(f"c{i}a")
                    nc.vector.tensor_tensor(c01[:], others[0][:], others[1][:], op=ALU.add)
                    ci = vtile(f"c{i}")
                    nc.vector.tensor_tensor(ci[:], c01[:], others[2][:], op=ALU.add)
                    cnt.append(ci)
                m01 = vtile("m01")
                nc.vector.tensor_tensor(m01[:], cnt[0][:], cnt[1][:], op=ALU.max)
                m23 = vtile("m23")
                nc.vector.tensor_tensor(m23[:], cnt[2][:], cnt[3][:], op=ALU.max)
                mx3 = vtile("mx3")
                nc.vector.tensor_tensor(mx3[:], m01[:], m23[:], op=ALU.max)
                ism = []
                for i in range(M):
                    e = vtile(f"ism{i}")
                    nc.vector.tensor_tensor(e[:], cnt[i][:], mx3[:], op=ALU.is_equal)
                    ism.append(e)
                s01 = vtile("s01")
                nc.vector.tensor_tensor(s01[:], ism[0][:], ism[1][:], op=ALU.add)
                s23 = vtile("s23")
                nc.vector.tensor_tensor(s23[:], ism[2][:], ism[3][:], op=ALU.add)
                s = vtile("s")
                nc.vector.tensor_tensor(s[:], s01[:], s23[:], op=ALU.add)
                mxp1 = vtile("mxp1")
                nc.vector.tensor_scalar(mxp1[:], mx3[:], 1.0, None, op0=ALU.add)
                tie = vtile("tie")
                nc.vector.tensor_tensor(tie[:], s[:], mxp1[:], op=ALU.not_equal)
                notie = vtile("notie")
                nc.vector.tensor_scalar(
                    notie[:], tie[:], -1.0, 1.0, op0=ALU.mult, op1=ALU.add
                )
                tie4 = vtile("tie4")
                nc.vector.tensor_scalar(tie4[:], tie[:], 0.25, None, op0=ALU.mult)
                us = []
                for i in range(M):
                    u = vtile(f"u{i}")
                    nc.vector.scalar_tensor_tensor(
                        u[:], rm[i][:], 1.0, ism[i][:], op0=ALU.add, op1=ALU.mult
                    )
                    us.append(u)
                b01 = vtile("b01")
                nc.vector.tensor_tensor(b01[:], us[0][:], us[1][:], op=ALU.max)
                b23 = vtile("b23")
                nc.vector.tensor_tensor(b23[:], us[2][:], us[3][:], op=ALU.max)
                b = vtile("b")
                nc.vector.tensor_tensor(b[:], b01[:], b23[:], op=ALU.max)
                W = vpool.tile([P, GW], F32, tag="W", bufs=2)
                for i in range(M):
                    oh = vtile(f"oh{i}")
                    nc.vector.tensor_tensor(oh[:], us[i][:], b[:], op=ALU.is_equal)
                    ohn = vtile(f"ohn{i}")
                    nc.vector.tensor_tensor(ohn[:], oh[:], notie[:], op=ALU.mult)
                    nc.vector.tensor_tensor(
                        W[:, i * G : (i + 1) * G], ohn[:], tie4[:], op=ALU.add
                    )

                # ---- weighted sum via PE diag matmuls (bf16)
                for j in range(G):
                    t = g * G + j
                    diags = []
                    for m in range(M):
                        d = wpool.tile([P, P], BF16, tag="diag", bufs=8)
                        nc.scalar.activation(
                            d[:], ident[:], AF.Copy,
                            scale=W[:, m * G + j : m * G + j + 1],
                        )
                        diags.append(d)
                    ps = ppool.tile([P, C], F32, tag="ps")
                    for lo, hi in ((0, 512), (512, C)):
                        for m in range(M):
                            nc.tensor.matmul(
                                ps[:, lo:hi], diags[m][:], xb[j][m][:, lo:hi],
                                start=(m == 0), stop=(m == M - 1),
                            )
                    ot = opool.tile([P, C], F32, tag="ot", bufs=3)
                    nc.scalar.activation(ot[:], ps[:], AF.Copy)
                    nc.sync.dma_start(out_ext[t * P : (t + 1) * P, :], ot[:])

    _split_multi_waits(nc)
    return nc


_GRAPH = None


def _get_graph():
    global _GRAPH
    if _GRAPH is None:
        _GRAPH = _build_graph()
    return _GRAPH


def _tie_break_randoms():
    """Reproduce the reference's jax.random.uniform(key(42), (4, N)).

    Must run on the same jax backend as the reference: the neuron backend's
    PRNG stream differs from the CPU backend's.
    """
    import jax

    r = np.asarray(jax.random.uniform(jax.random.key(42), (M, N), dtype="float32"))
    # core layout: r_core[p, g*GW + m*G + j] = r[m, core*NS + (g*G+j)*P + p]
    rc = r.reshape(M, N_CORES, NG, G, P)          # [m, core, g, j, p]
    rc = rc.transpose(1, 4, 2, 0, 3)              # [core, p, g, m, j]
    return np.ascontiguousarray(rc.reshape(N_CORES, P, NG * GW).astype(np.float32))


def kernel(outputs1, outputs2, outputs3, outputs4):
    xs = [
        np.ascontiguousarray(np.asarray(o, dtype=np.float32))
        for o in (outputs1, outputs2, outputs3, outputs4)
    ]
    rcs = _tie_break_randoms()
    in_maps = []
    for c in range(N_CORES):
        im = {f"x{m}": xs[m][c * NS : (c + 1) * NS] for m in range(M)}
        im["r"] = rcs[c]
        in_maps.append(im)

    nc = _get_graph()
    from concourse.bass_utils import run_bass_kernel_spmd

    res = run_bass_kernel_spmd(nc, in_maps, core_ids=list(range(N_CORES)))
    return np.concatenate([res.results[c]["out"] for c in range(N_CORES)], axis=0)
